# revision 12
# baseline (speedup 1.0000x reference)
"""DMoN (graph pooling) kernel for 8 Trainium2 NeuronCores.

Full inputs -> full outputs. Internally:
  - nodes sharded 8 ways (12500/core); edges sharded by dst-owner core
  - phase A (per core): S = softmax(F@W+b) on own rows; U = S^T F partial
  - AllGather of S^T (f32) across cores
  - phase B (per core): edge reductions tg = sum w*<S[src],S[dst]>,
    v = sum w*S[dst] via GPSIMD ap_gather on SBUF-resident S tables
  - host: tiny final reductions (K=16 / KxD matrices and scalars)

Outputs mirror reference(): (features_pooled, assignments, spectral_loss, collapse_loss)
"""
import sys

sys.path.insert(0, "/opt/trn_rl_repo")
sys.path.insert(0, "/opt/trn_rl_repo/concourse")

import numpy as np
import ml_dtypes

from concourse import bacc, bass, mybir
from concourse.tile import TileContext
from concourse.bass_utils import run_bass_kernel_spmd
from concourse.masks import make_identity

NCORES = 8
N = 100000
E = 3200000
D = 384
K = 16
NPC = N // NCORES          # 12500 nodes per core
NTILES = (NPC + 127) // 128  # 98
NPAD = NTILES * 128        # 12544
EPC = E // NCORES          # 400000 edges per core
NI_CHUNK = 6400            # ap_gather indices per core-group per call
# per-group stream length, padded. 8 groups; expected 50000; slack for imbalance
L_STREAM = 52800           # multiple of NI_CHUNK*... (52800 = 16.5 chunks) -> use 53760?
# make it a multiple of NI_CHUNK and 16:
L_STREAM = ((EPC // 8 + 4000 + NI_CHUNK - 1) // NI_CHUNK) * NI_CHUNK  # 54400->  17 chunks
N_CHUNKS = L_STREAM // NI_CHUNK

F32 = mybir.dt.float32
BF16 = mybir.dt.bfloat16
I16 = mybir.dt.int16


def build_kernel():
    nc = bacc.Bacc("TRN2", debug=False, num_devices=NCORES)

    ft = nc.dram_tensor("ft", (D, NPAD), F32, kind="ExternalInput")       # F^T shard
    fn = nc.dram_tensor("fn", (NPAD, D), F32, kind="ExternalInput")       # F shard
    wmat = nc.dram_tensor("wmat", (D + 1, K), F32, kind="ExternalInput")  # [W; b]
    esrc = nc.dram_tensor("esrc", (128, L_STREAM // 16), I16, kind="ExternalInput")
    edst = nc.dram_tensor("edst", (128, L_STREAM // 16), I16, kind="ExternalInput")
    wexp = nc.dram_tensor("wexp", (128, L_STREAM), BF16, kind="ExternalInput")

    s_out = nc.dram_tensor("s_out", (NPAD, K), F32, kind="ExternalOutput")
    u_out = nc.dram_tensor("u_out", (K, D), F32, kind="ExternalOutput")
    pv = nc.dram_tensor("pv", (128, 1), F32, kind="ExternalOutput")
    ptg = nc.dram_tensor("ptg", (128, 1), F32, kind="ExternalOutput")

    cc_in = nc.dram_tensor("cc_in", (K, NPAD), BF16)                      # local S^T
    cc_out = nc.dram_tensor("cc_out", (NCORES * K, NPAD), BF16, addr_space="Shared")

    with TileContext(nc) as tc:
        # ---------------- Phase A ----------------
        with tc.tile_pool(name="pa", bufs=3) as pa, \
             tc.tile_pool(name="pa1", bufs=1) as pa1, \
             tc.tile_pool(name="papsum", bufs=3, space="PSUM") as pap, \
             tc.tile_pool(name="paupsum", bufs=1, space="PSUM") as paup:
            wsb = pa1.tile([128, 3 * K], F32)
            for c in range(3):
                nc.sync.dma_start(out=wsb[:, c * K:(c + 1) * K],
                                  in_=wmat.ap()[c * 128:(c + 1) * 128, :])
            brow = pa1.tile([1, K], F32)
            nc.sync.dma_start(out=brow[:, :], in_=wmat.ap()[D:D + 1, :])
            onesb = pa1.tile([1, 128], F32)
            nc.vector.memset(onesb[:, :], 1.0)
            ident = pa1.tile([128, 128], F32)
            make_identity(nc, ident)

            u_psum = paup.tile([K, D], F32, space="PSUM")
            s_all = pa1.tile([128, NTILES * K], F32)   # staging for s_out

            for i in range(NTILES):
                ftl = pa.tile([128, 3 * 128], F32, tag="ftl")
                for c in range(3):
                    nc.sync.dma_start(
                        out=ftl[:, c * 128:(c + 1) * 128],
                        in_=ft.ap()[c * 128:(c + 1) * 128, i * 128:(i + 1) * 128])
                fnl = pa.tile([128, D], F32, tag="fnl")
                nc.sync.dma_start(out=fnl[:, :], in_=fn.ap()[i * 128:(i + 1) * 128, :])

                logits = pap.tile([128, K], F32, space="PSUM", tag="logits")
                for c in range(3):
                    nc.tensor.matmul(logits[:, :], lhsT=ftl[:, c * 128:(c + 1) * 128],
                                     rhs=wsb[:, c * K:(c + 1) * K],
                                     start=(c == 0), stop=False)
                nc.tensor.matmul(logits[:, :], lhsT=onesb[:1, :], rhs=brow[:1, :],
                                 start=False, stop=True)

                # logits are O(+-6) for this problem scale: exp is safe in f32
                # without the max-subtraction trick.
                e_t = pa.tile([128, K], F32, tag="e_t")
                ssum = pa.tile([128, 1], F32, tag="ssum")
                nc.scalar.activation(e_t[:, :], logits[:, :],
                                     mybir.ActivationFunctionType.Exp,
                                     scale=1.0, accum_out=ssum[:, :1])
                rsum = pa.tile([128, 1], F32, tag="rsum")
                nc.vector.reciprocal(rsum[:, :], ssum[:, :])
                s_t = pa.tile([128, K], F32, tag="s_t")
                nc.vector.tensor_scalar_mul(s_t[:, :], e_t[:, :], rsum[:, :1])

                # stage S rows for one big DMA later
                nc.vector.tensor_copy(s_all[:, i * K:(i + 1) * K], s_t[:, :])

                # U += S_tile^T @ F_tile
                nc.tensor.matmul(u_psum[:, :], lhsT=s_t[:, :], rhs=fnl[:, :],
                                 start=(i == 0), stop=(i == NTILES - 1))

                # S^T chunk -> cc_in
                st_ps = pap.tile([K, 128], F32, space="PSUM", tag="st_ps")
                nc.tensor.transpose(out=st_ps[:, :], in_=s_t[:, :], identity=ident[:, :])
                st_sb = pa.tile([K, 128], BF16, tag="st_sb")
                nc.vector.tensor_copy(st_sb[:, :], st_ps[:, :])
                nc.sync.dma_start(out=cc_in.ap()[:, i * 128:(i + 1) * 128],
                                  in_=st_sb[:, :])

            u_sb = pa1.tile([K, D], F32)
            nc.vector.tensor_copy(u_sb[:, :], u_psum[:, :])
            nc.sync.dma_start(out=u_out.ap()[:, :], in_=u_sb[:, :])
            # s_out: [NPAD, K] viewed [NTILES, 128, K]; s_all is [128, NTILES, K]
            nc.sync.dma_start(
                out=s_out.ap().rearrange("(t p) k -> p t k", p=128)[:, :, :],
                in_=s_all[:, :].rearrange("p (t k) -> p t k", k=K)[:, :, :])

            nc.gpsimd.collective_compute(
                "AllGather", mybir.AluOpType.bypass,
                replica_groups=[list(range(NCORES))],
                ins=[cc_in.ap()[:, :]],
                outs=[cc_out.ap()[:, :]],
            )

        # ---------------- Phase B ----------------
        with tc.tile_pool(name="pb", bufs=2) as pb, \
             tc.tile_pool(name="pbg", bufs=1) as pbg, \
             tc.tile_pool(name="pb1", bufs=1) as pb1:
            # bf16 over the wire; ap_gather tables must be f32 -> cast in chunks
            sgrp = pb1.tile([128, NPAD], F32)   # src table: group g -> S^T of group g
            srep = pb1.tile([128, NPAD], F32)   # dst table: own S^T replicated x8
            CSTCH = NPAD // 4
            for q in range(4):
                stg = pb.tile([128, CSTCH], BF16, tag="stg")
                nc.sync.dma_start(out=stg[:, :],
                                  in_=cc_out.ap()[:, q * CSTCH:(q + 1) * CSTCH])
                nc.vector.tensor_copy(sgrp[:, q * CSTCH:(q + 1) * CSTCH], stg[:, :])
            for q in range(4):
                stg = pb.tile([128, CSTCH], BF16, tag="stg")
                for g in range(NCORES):
                    nc.sync.dma_start(out=stg[g * K:(g + 1) * K, :],
                                      in_=cc_in.ap()[:, q * CSTCH:(q + 1) * CSTCH])
                nc.vector.tensor_copy(srep[:, q * CSTCH:(q + 1) * CSTCH], stg[:, :])

            acc_v = pb1.tile([128, 1], F32)
            acc_t = pb1.tile([128, 1], F32)
            nc.vector.memset(acc_v[:, :], 0.0)
            nc.vector.memset(acc_t[:, :], 0.0)

            for ch in range(N_CHUNKS):
                i0 = ch * (NI_CHUNK // 16)
                i1 = (ch + 1) * (NI_CHUNK // 16)
                ix_s = pb.tile([128, NI_CHUNK // 16], I16, tag="ix_s")
                nc.sync.dma_start(out=ix_s[:, :], in_=esrc.ap()[:, i0:i1])
                ix_d = pb.tile([128, NI_CHUNK // 16], I16, tag="ix_d")
                nc.sync.dma_start(out=ix_d[:, :], in_=edst.ap()[:, i0:i1])
                wch = pb.tile([128, NI_CHUNK], BF16, tag="wch")
                nc.sync.dma_start(out=wch[:, :],
                                  in_=wexp.ap()[:, ch * NI_CHUNK:(ch + 1) * NI_CHUNK])

                gs = pbg.tile([128, NI_CHUNK], F32, tag="gs")
                nc.gpsimd.ap_gather(gs[:, :], sgrp[:, :], ix_s[:, :],
                                    channels=128, num_elems=NPAD, d=1,
                                    num_idxs=NI_CHUNK)
                gd = pbg.tile([128, NI_CHUNK], F32, tag="gd")
                nc.gpsimd.ap_gather(gd[:, :], srep[:, :], ix_d[:, :],
                                    channels=128, num_elems=NPAD, d=1,
                                    num_idxs=NI_CHUNK)

                # in-place: gd <- gd*w ; gs <- gs*gd ; then per-chunk reductions
                nc.vector.tensor_tensor(out=gd[:, :], in0=gd[:, :], in1=wch[:, :],
                                        op=mybir.AluOpType.mult)
                rv = pb.tile([128, 1], F32, tag="rv")
                nc.vector.tensor_reduce(rv[:, :], gd[:, :], axis=mybir.AxisListType.X,
                                        op=mybir.AluOpType.add)
                nc.vector.tensor_tensor(out=acc_v[:, :], in0=acc_v[:, :], in1=rv[:, :],
                                        op=mybir.AluOpType.add)
                nc.vector.tensor_tensor(out=gs[:, :], in0=gs[:, :], in1=gd[:, :],
                                        op=mybir.AluOpType.mult)
                rt = pb.tile([128, 1], F32, tag="rt")
                nc.vector.tensor_reduce(rt[:, :], gs[:, :], axis=mybir.AxisListType.X,
                                        op=mybir.AluOpType.add)
                nc.vector.tensor_tensor(out=acc_t[:, :], in0=acc_t[:, :], in1=rt[:, :],
                                        op=mybir.AluOpType.add)

            nc.sync.dma_start(out=pv.ap()[:, :], in_=acc_v[:, :])
            nc.sync.dma_start(out=ptg.ap()[:, :], in_=acc_t[:, :])

    nc.compile()
    return nc


_NC_CACHE = None
_RUNNER_CACHE = None


def _get_nc():
    global _NC_CACHE
    if _NC_CACHE is None:
        _NC_CACHE = build_kernel()
    return _NC_CACHE


def _make_runner(nc):
    """Reusable jitted SPMD runner (mirrors bass2jax.run_bass_via_pjrt, but
    caches the jit across calls and accepts pre-uploaded device arrays)."""
    import jax
    from jax.sharding import Mesh, PartitionSpec
    from jax.experimental.shard_map import shard_map
    from concourse import bass2jax, mybir as mb
    from concourse.bass2jax import _bass_exec_p, partition_id_tensor

    bass2jax.install_neuronx_cc_hook()

    partition_name = nc.partition_id_tensor.name if nc.partition_id_tensor else None
    in_names, out_names, out_avals, zero_shapes = [], [], [], []
    for alloc in nc.m.functions[0].allocations:
        if not isinstance(alloc, mb.MemoryLocationSet):
            continue
        name = alloc.memorylocations[0].name
        if alloc.kind == "ExternalInput":
            if name != partition_name:
                in_names.append(name)
        elif alloc.kind == "ExternalOutput":
            out_names.append(name)
            shape = tuple(alloc.tensor_shape)
            dtype = mb.dt.np(alloc.dtype)
            out_avals.append(jax.core.ShapedArray(shape, dtype))
            zero_shapes.append((shape, dtype))
    n_params = len(in_names)
    n_outs = len(out_avals)
    all_in_names = list(in_names) + list(out_names)
    if partition_name is not None:
        all_in_names.append(partition_name)
    donate = tuple(range(n_params, n_params + n_outs))

    def _body(*args):
        operands = list(args)
        if partition_name is not None:
            operands.append(partition_id_tensor())
        outs = _bass_exec_p.bind(
            *operands,
            out_avals=tuple(out_avals),
            in_names=tuple(all_in_names),
            out_names=tuple(out_names),
            lowering_input_output_aliases=(),
            sim_require_finite=True,
            sim_require_nnan=True,
            nc=nc,
        )
        return tuple(outs)

    devices = jax.devices()[:NCORES]
    mesh = Mesh(np.asarray(devices), ("core",))
    sharded = jax.jit(
        shard_map(_body, mesh=mesh,
                  in_specs=(PartitionSpec("core"),) * (n_params + n_outs),
                  out_specs=(PartitionSpec("core"),) * n_outs,
                  check_rep=False),
        donate_argnums=donate, keep_unused=True,
    )

    def concat_inputs(in_maps):
        return [np.concatenate([np.asarray(m[name]) for m in in_maps], axis=0)
                for name in in_names]

    def make_zeros():
        return [np.zeros((NCORES * s[0], *s[1:]), d) for s, d in zero_shapes]

    def run(concat_in):
        out_arrs = sharded(*concat_in, *make_zeros())
        import jax as _jax
        _jax.block_until_ready(out_arrs)
        return [
            {name: np.asarray(out_arrs[i]).reshape(NCORES, *out_avals[i].shape)[c]
             for i, name in enumerate(out_names)}
            for c in range(NCORES)
        ]

    run.concat_inputs = concat_inputs
    run.make_zeros = make_zeros
    run.sharded = sharded
    return run


def _get_runner():
    global _RUNNER_CACHE
    if _RUNNER_CACHE is None:
        _RUNNER_CACHE = _make_runner(_get_nc())
    return _RUNNER_CACHE


def _prep_inputs(features, edge_src, edge_dst, edge_weight, W, b):
    features = np.asarray(features, dtype=np.float32)
    edge_src = np.asarray(edge_src).astype(np.int64)
    edge_dst = np.asarray(edge_dst).astype(np.int64)
    edge_weight = np.asarray(edge_weight, dtype=np.float32)
    W = np.asarray(W, dtype=np.float32)
    b = np.asarray(b, dtype=np.float32)

    wmat = np.concatenate([W, b[None, :]], axis=0)  # [385, 16]

    in_maps = []
    FT = np.ascontiguousarray(features.T)  # [384, 100000]
    owner = edge_dst // NPC                # dst-owner core per edge
    for c in range(NCORES):
        r0, r1 = c * NPC, (c + 1) * NPC
        ftc = np.zeros((D, NPAD), dtype=np.float32)
        ftc[:, :NPC] = FT[:, r0:r1]
        fnc = np.zeros((NPAD, D), dtype=np.float32)
        fnc[:NPC, :] = features[r0:r1, :]

        sel = owner == c
        es = edge_src[sel]
        ed = edge_dst[sel]
        ew = edge_weight[sel]
        # bucket by src group
        g_s = es // NPC
        src_loc = (es % NPC).astype(np.int16)
        dst_loc = (ed % NPC).astype(np.int16)

        esrc = np.zeros((128, L_STREAM // 16), dtype=np.int16)
        edst = np.zeros((128, L_STREAM // 16), dtype=np.int16)
        wexp = np.zeros((128, L_STREAM), dtype=ml_dtypes.bfloat16)
        for g in range(NCORES):
            gsel = g_s == g
            n_g = int(gsel.sum())
            assert n_g <= L_STREAM, f"bucket overflow core {c} group {g}: {n_g}"
            sl = np.zeros(L_STREAM, dtype=np.int16)
            dl = np.zeros(L_STREAM, dtype=np.int16)
            wl = np.zeros(L_STREAM, dtype=np.float32)
            sl[:n_g] = src_loc[gsel]
            dl[:n_g] = dst_loc[gsel]
            wl[:n_g] = ew[gsel]
            # wrap [L] -> [L//16, 16] -> partitions 16g..16g+16 hold transposed
            esrc[16 * g:16 * (g + 1), :] = sl.reshape(-1, 16).T
            edst[16 * g:16 * (g + 1), :] = dl.reshape(-1, 16).T
            wexp[16 * g:16 * (g + 1), :] = np.broadcast_to(
                wl[None, :], (16, L_STREAM)).astype(ml_dtypes.bfloat16)

        in_maps.append({
            "ft": ftc, "fn": fnc, "wmat": wmat,
            "esrc": esrc, "edst": edst, "wexp": wexp,
        })
    return in_maps


def _selu(x):
    alpha = 1.6732632423543772848170429916717
    scale = 1.0507009873554804934193349852946
    return (scale * np.where(x > 0, x, alpha * (np.exp(x) - 1))).astype(np.float32)


def _postprocess(results):
    S = np.concatenate([results[c]["s_out"][:NPC] for c in range(NCORES)], axis=0)
    U = np.sum([results[c]["u_out"].astype(np.float64) for c in range(NCORES)], axis=0)
    pv_all = np.sum([results[c]["pv"].astype(np.float64) for c in range(NCORES)],
                    axis=0)[:, 0]
    tg = float(np.sum([results[c]["ptg"].astype(np.float64) for c in range(NCORES)]))

    v = pv_all.reshape(8, 16).sum(axis=0)            # [16]
    m = float(v.sum())                               # sum of edge weights
    cs = S.astype(np.float64).sum(axis=0)            # cluster sizes [16]

    features_pooled = _selu((U / cs[:, None]).astype(np.float32))
    spectral = np.float32(-(tg - float(v @ v) / (2.0 * m)) / (2.0 * m))
    collapse = np.float32(0.1 * (np.linalg.norm(cs) / N * np.sqrt(K) - 1.0))
    return features_pooled, S.astype(np.float32), spectral, collapse


def kernel(features, edge_src, edge_dst, edge_weight, W, b):
    runner = _get_runner()
    in_maps = _prep_inputs(features, edge_src, edge_dst, edge_weight, W, b)
    results = runner(runner.concat_inputs(in_maps))
    return _postprocess(results)


# revision 13
# speedup vs baseline: 10.7960x; 10.7960x over previous
"""DMoN (graph pooling) kernel for 8 Trainium2 NeuronCores.

Full inputs -> full outputs. Internally:
  - nodes sharded 8 ways (12500/core); edges sharded by dst-owner core
  - phase A (per core): S = softmax(F@W+b) on own rows; U = S^T F partial
  - AllGather of S^T (f32) across cores
  - phase B (per core): edge reductions tg = sum w*<S[src],S[dst]>,
    v = sum w*S[dst] via GPSIMD ap_gather on SBUF-resident S tables
  - host: tiny final reductions (K=16 / KxD matrices and scalars)

Outputs mirror reference(): (features_pooled, assignments, spectral_loss, collapse_loss)
"""
import sys

sys.path.insert(0, "/opt/trn_rl_repo")
sys.path.insert(0, "/opt/trn_rl_repo/concourse")

import numpy as np
import ml_dtypes

from concourse import bacc, bass, mybir
from concourse.tile import TileContext
from concourse.bass_utils import run_bass_kernel_spmd
from concourse.masks import make_identity

NCORES = 8
N = 100000
E = 3200000
D = 384
K = 16
NPC = N // NCORES          # 12500 nodes per core
NTILES = (NPC + 127) // 128  # 98
NPAD = NTILES * 128        # 12544
EPC = E // NCORES          # 400000 edges per core
NI_CHUNK = 3200            # ap_gather indices per core-group per call
# per-group stream length, padded. 8 groups; expected 50000; slack for imbalance
L_STREAM = 52800           # multiple of NI_CHUNK*... (52800 = 16.5 chunks) -> use 53760?
# make it a multiple of NI_CHUNK and 16:
L_STREAM = ((EPC // 8 + 4000 + NI_CHUNK - 1) // NI_CHUNK) * NI_CHUNK  # 54400->  17 chunks
N_CHUNKS = L_STREAM // NI_CHUNK

F32 = mybir.dt.float32
BF16 = mybir.dt.bfloat16
I16 = mybir.dt.int16


def build_kernel():
    nc = bacc.Bacc("TRN2", debug=False, num_devices=NCORES)

    ft = nc.dram_tensor("ft", (D, NPAD), F32, kind="ExternalInput")       # F^T shard
    fn = nc.dram_tensor("fn", (NPAD, D), F32, kind="ExternalInput")       # F shard
    wmat = nc.dram_tensor("wmat", (D + 1, K), F32, kind="ExternalInput")  # [W; b]
    esrc = nc.dram_tensor("esrc", (128, L_STREAM // 16), I16, kind="ExternalInput")
    edst = nc.dram_tensor("edst", (128, L_STREAM // 16), I16, kind="ExternalInput")
    wexp = nc.dram_tensor("wexp", (128, L_STREAM), BF16, kind="ExternalInput")

    s_out = nc.dram_tensor("s_out", (NPAD, K), F32, kind="ExternalOutput")
    u_out = nc.dram_tensor("u_out", (K, D), F32, kind="ExternalOutput")
    pv = nc.dram_tensor("pv", (128, 1), F32, kind="ExternalOutput")
    ptg = nc.dram_tensor("ptg", (128, 1), F32, kind="ExternalOutput")

    cc_in = nc.dram_tensor("cc_in", (K, NPAD), BF16)                      # local S^T
    cc_out = nc.dram_tensor("cc_out", (NCORES * K, NPAD), BF16, addr_space="Shared")

    with TileContext(nc) as tc:
        # ---------------- Phase A ----------------
        with tc.tile_pool(name="pa", bufs=3) as pa, \
             tc.tile_pool(name="pa1", bufs=1) as pa1, \
             tc.tile_pool(name="papsum", bufs=3, space="PSUM") as pap, \
             tc.tile_pool(name="paupsum", bufs=1, space="PSUM") as paup:
            wsb = pa1.tile([128, 3 * K], F32)
            for c in range(3):
                nc.sync.dma_start(out=wsb[:, c * K:(c + 1) * K],
                                  in_=wmat.ap()[c * 128:(c + 1) * 128, :])
            brow = pa1.tile([1, K], F32)
            nc.sync.dma_start(out=brow[:, :], in_=wmat.ap()[D:D + 1, :])
            onesb = pa1.tile([1, 128], F32)
            nc.vector.memset(onesb[:, :], 1.0)
            ident = pa1.tile([128, 128], F32)
            make_identity(nc, ident)

            u_psum = paup.tile([K, D], F32, space="PSUM")
            s_all = pa1.tile([128, NTILES * K], F32)   # staging for s_out

            for i in range(NTILES):
                ftl = pa.tile([128, 3 * 128], F32, tag="ftl")
                for c in range(3):
                    nc.sync.dma_start(
                        out=ftl[:, c * 128:(c + 1) * 128],
                        in_=ft.ap()[c * 128:(c + 1) * 128, i * 128:(i + 1) * 128])
                fnl = pa.tile([128, D], F32, tag="fnl")
                nc.sync.dma_start(out=fnl[:, :], in_=fn.ap()[i * 128:(i + 1) * 128, :])

                logits = pap.tile([128, K], F32, space="PSUM", tag="logits")
                for c in range(3):
                    nc.tensor.matmul(logits[:, :], lhsT=ftl[:, c * 128:(c + 1) * 128],
                                     rhs=wsb[:, c * K:(c + 1) * K],
                                     start=(c == 0), stop=False)
                nc.tensor.matmul(logits[:, :], lhsT=onesb[:1, :], rhs=brow[:1, :],
                                 start=False, stop=True)

                # logits are O(+-6) for this problem scale: exp is safe in f32
                # without the max-subtraction trick.
                e_t = pa.tile([128, K], F32, tag="e_t")
                ssum = pa.tile([128, 1], F32, tag="ssum")
                nc.scalar.activation(e_t[:, :], logits[:, :],
                                     mybir.ActivationFunctionType.Exp,
                                     scale=1.0, accum_out=ssum[:, :1])
                rsum = pa.tile([128, 1], F32, tag="rsum")
                nc.vector.reciprocal(rsum[:, :], ssum[:, :])
                s_t = pa.tile([128, K], F32, tag="s_t")
                nc.vector.tensor_scalar_mul(s_t[:, :], e_t[:, :], rsum[:, :1])

                # stage S rows for one big DMA later
                nc.vector.tensor_copy(s_all[:, i * K:(i + 1) * K], s_t[:, :])

                # U += S_tile^T @ F_tile
                nc.tensor.matmul(u_psum[:, :], lhsT=s_t[:, :], rhs=fnl[:, :],
                                 start=(i == 0), stop=(i == NTILES - 1))

                # S^T chunk -> cc_in
                st_ps = pap.tile([K, 128], F32, space="PSUM", tag="st_ps")
                nc.tensor.transpose(out=st_ps[:, :], in_=s_t[:, :], identity=ident[:, :])
                st_sb = pa.tile([K, 128], BF16, tag="st_sb")
                nc.vector.tensor_copy(st_sb[:, :], st_ps[:, :])
                nc.sync.dma_start(out=cc_in.ap()[:, i * 128:(i + 1) * 128],
                                  in_=st_sb[:, :])

            u_sb = pa1.tile([K, D], F32)
            nc.vector.tensor_copy(u_sb[:, :], u_psum[:, :])
            nc.sync.dma_start(out=u_out.ap()[:, :], in_=u_sb[:, :])
            # s_out: [NPAD, K] viewed [NTILES, 128, K]; s_all is [128, NTILES, K]
            nc.sync.dma_start(
                out=s_out.ap().rearrange("(t p) k -> p t k", p=128)[:, :, :],
                in_=s_all[:, :].rearrange("p (t k) -> p t k", k=K)[:, :, :])

            nc.gpsimd.collective_compute(
                "AllGather", mybir.AluOpType.bypass,
                replica_groups=[list(range(NCORES))],
                ins=[cc_in.ap()[:, :]],
                outs=[cc_out.ap()[:, :]],
            )

        # ---------------- Phase B ----------------
        with tc.tile_pool(name="pb", bufs=2) as pb, \
             tc.tile_pool(name="pb1", bufs=1) as pb1:
            # bf16 over the wire; ap_gather tables must be f32 -> cast in chunks
            sgrp = pb1.tile([128, NPAD], F32)   # src table: group g -> S^T of group g
            srep = pb1.tile([128, NPAD], F32)   # dst table: own S^T replicated x8
            CSTCH = NPAD // 4
            for q in range(4):
                stg = pb.tile([128, CSTCH], BF16, tag="stg")
                nc.sync.dma_start(out=stg[:, :],
                                  in_=cc_out.ap()[:, q * CSTCH:(q + 1) * CSTCH])
                nc.vector.tensor_copy(sgrp[:, q * CSTCH:(q + 1) * CSTCH], stg[:, :])
            for q in range(4):
                stg = pb.tile([128, CSTCH], BF16, tag="stg")
                for g in range(NCORES):
                    nc.sync.dma_start(out=stg[g * K:(g + 1) * K, :],
                                      in_=cc_in.ap()[:, q * CSTCH:(q + 1) * CSTCH])
                nc.vector.tensor_copy(srep[:, q * CSTCH:(q + 1) * CSTCH], stg[:, :])

            acc_v = pb1.tile([128, NI_CHUNK], F32)
            acc_t = pb1.tile([128, NI_CHUNK], F32)
            nc.vector.memset(acc_v[:, :], 0.0)
            nc.vector.memset(acc_t[:, :], 0.0)

            for ch in range(N_CHUNKS):
                i0 = ch * (NI_CHUNK // 16)
                i1 = (ch + 1) * (NI_CHUNK // 16)
                ix_s = pb.tile([128, NI_CHUNK // 16], I16, tag="ix_s")
                nc.sync.dma_start(out=ix_s[:, :], in_=esrc.ap()[:, i0:i1])
                ix_d = pb.tile([128, NI_CHUNK // 16], I16, tag="ix_d")
                nc.sync.dma_start(out=ix_d[:, :], in_=edst.ap()[:, i0:i1])
                wch = pb.tile([128, NI_CHUNK], BF16, tag="wch")
                nc.sync.dma_start(out=wch[:, :],
                                  in_=wexp.ap()[:, ch * NI_CHUNK:(ch + 1) * NI_CHUNK])

                gs = pb.tile([128, NI_CHUNK], F32, tag="gs")
                nc.gpsimd.ap_gather(gs[:, :], sgrp[:, :], ix_s[:, :],
                                    channels=128, num_elems=NPAD, d=1,
                                    num_idxs=NI_CHUNK)
                gd = pb.tile([128, NI_CHUNK], F32, tag="gd")
                nc.gpsimd.ap_gather(gd[:, :], srep[:, :], ix_d[:, :],
                                    channels=128, num_elems=NPAD, d=1,
                                    num_idxs=NI_CHUNK)

                # in-place: gd <- gd*w ; acc_v += gd ; gs <- gs*gd ; acc_t += gs
                nc.vector.tensor_tensor(out=gd[:, :], in0=gd[:, :], in1=wch[:, :],
                                        op=mybir.AluOpType.mult)
                nc.vector.tensor_tensor(out=acc_v[:, :], in0=acc_v[:, :], in1=gd[:, :],
                                        op=mybir.AluOpType.add)
                nc.vector.tensor_tensor(out=gs[:, :], in0=gs[:, :], in1=gd[:, :],
                                        op=mybir.AluOpType.mult)
                nc.vector.tensor_tensor(out=acc_t[:, :], in0=acc_t[:, :], in1=gs[:, :],
                                        op=mybir.AluOpType.add)

            pv_sb = pb1.tile([128, 1], F32)
            nc.vector.tensor_reduce(pv_sb[:, :], acc_v[:, :],
                                    axis=mybir.AxisListType.X, op=mybir.AluOpType.add)
            nc.sync.dma_start(out=pv.ap()[:, :], in_=pv_sb[:, :])
            ptg_sb = pb1.tile([128, 1], F32)
            nc.vector.tensor_reduce(ptg_sb[:, :], acc_t[:, :],
                                    axis=mybir.AxisListType.X, op=mybir.AluOpType.add)
            nc.sync.dma_start(out=ptg.ap()[:, :], in_=ptg_sb[:, :])

    nc.compile()
    return nc


_NC_CACHE = None
_RUNNER_CACHE = None


def _get_nc():
    global _NC_CACHE
    if _NC_CACHE is None:
        _NC_CACHE = build_kernel()
    return _NC_CACHE


def _make_runner(nc):
    """Reusable jitted SPMD runner (mirrors bass2jax.run_bass_via_pjrt, but
    caches the jit across calls and accepts pre-uploaded device arrays)."""
    import jax
    from jax.sharding import Mesh, PartitionSpec
    from jax.experimental.shard_map import shard_map
    from concourse import bass2jax, mybir as mb
    from concourse.bass2jax import _bass_exec_p, partition_id_tensor

    bass2jax.install_neuronx_cc_hook()

    partition_name = nc.partition_id_tensor.name if nc.partition_id_tensor else None
    in_names, out_names, out_avals, zero_shapes = [], [], [], []
    for alloc in nc.m.functions[0].allocations:
        if not isinstance(alloc, mb.MemoryLocationSet):
            continue
        name = alloc.memorylocations[0].name
        if alloc.kind == "ExternalInput":
            if name != partition_name:
                in_names.append(name)
        elif alloc.kind == "ExternalOutput":
            out_names.append(name)
            shape = tuple(alloc.tensor_shape)
            dtype = mb.dt.np(alloc.dtype)
            out_avals.append(jax.core.ShapedArray(shape, dtype))
            zero_shapes.append((shape, dtype))
    n_params = len(in_names)
    n_outs = len(out_avals)
    all_in_names = list(in_names) + list(out_names)
    if partition_name is not None:
        all_in_names.append(partition_name)
    donate = tuple(range(n_params, n_params + n_outs))

    def _body(*args):
        operands = list(args)
        if partition_name is not None:
            operands.append(partition_id_tensor())
        outs = _bass_exec_p.bind(
            *operands,
            out_avals=tuple(out_avals),
            in_names=tuple(all_in_names),
            out_names=tuple(out_names),
            lowering_input_output_aliases=(),
            sim_require_finite=True,
            sim_require_nnan=True,
            nc=nc,
        )
        return tuple(outs)

    devices = jax.devices()[:NCORES]
    mesh = Mesh(np.asarray(devices), ("core",))
    sharded = jax.jit(
        shard_map(_body, mesh=mesh,
                  in_specs=(PartitionSpec("core"),) * (n_params + n_outs),
                  out_specs=(PartitionSpec("core"),) * n_outs,
                  check_rep=False),
        donate_argnums=donate, keep_unused=True,
    )

    def concat_inputs(in_maps):
        return [np.concatenate([np.asarray(m[name]) for m in in_maps], axis=0)
                for name in in_names]

    def make_zeros():
        return [np.zeros((NCORES * s[0], *s[1:]), d) for s, d in zero_shapes]

    def run(concat_in):
        out_arrs = sharded(*concat_in, *make_zeros())
        import jax as _jax
        _jax.block_until_ready(out_arrs)
        return [
            {name: np.asarray(out_arrs[i]).reshape(NCORES, *out_avals[i].shape)[c]
             for i, name in enumerate(out_names)}
            for c in range(NCORES)
        ]

    run.concat_inputs = concat_inputs
    run.make_zeros = make_zeros
    run.sharded = sharded
    return run


def _get_runner():
    global _RUNNER_CACHE
    if _RUNNER_CACHE is None:
        _RUNNER_CACHE = _make_runner(_get_nc())
    return _RUNNER_CACHE


def _prep_inputs(features, edge_src, edge_dst, edge_weight, W, b):
    features = np.asarray(features, dtype=np.float32)
    edge_src = np.asarray(edge_src).astype(np.int64)
    edge_dst = np.asarray(edge_dst).astype(np.int64)
    edge_weight = np.asarray(edge_weight, dtype=np.float32)
    W = np.asarray(W, dtype=np.float32)
    b = np.asarray(b, dtype=np.float32)

    wmat = np.concatenate([W, b[None, :]], axis=0)  # [385, 16]

    in_maps = []
    FT = np.ascontiguousarray(features.T)  # [384, 100000]
    owner = edge_dst // NPC                # dst-owner core per edge
    for c in range(NCORES):
        r0, r1 = c * NPC, (c + 1) * NPC
        ftc = np.zeros((D, NPAD), dtype=np.float32)
        ftc[:, :NPC] = FT[:, r0:r1]
        fnc = np.zeros((NPAD, D), dtype=np.float32)
        fnc[:NPC, :] = features[r0:r1, :]

        sel = owner == c
        es = edge_src[sel]
        ed = edge_dst[sel]
        ew = edge_weight[sel]
        # bucket by src group
        g_s = es // NPC
        src_loc = (es % NPC).astype(np.int16)
        dst_loc = (ed % NPC).astype(np.int16)

        esrc = np.zeros((128, L_STREAM // 16), dtype=np.int16)
        edst = np.zeros((128, L_STREAM // 16), dtype=np.int16)
        wexp = np.zeros((128, L_STREAM), dtype=ml_dtypes.bfloat16)
        for g in range(NCORES):
            gsel = g_s == g
            n_g = int(gsel.sum())
            assert n_g <= L_STREAM, f"bucket overflow core {c} group {g}: {n_g}"
            sl = np.zeros(L_STREAM, dtype=np.int16)
            dl = np.zeros(L_STREAM, dtype=np.int16)
            wl = np.zeros(L_STREAM, dtype=np.float32)
            sl[:n_g] = src_loc[gsel]
            dl[:n_g] = dst_loc[gsel]
            wl[:n_g] = ew[gsel]
            # wrap [L] -> [L//16, 16] -> partitions 16g..16g+16 hold transposed
            esrc[16 * g:16 * (g + 1), :] = sl.reshape(-1, 16).T
            edst[16 * g:16 * (g + 1), :] = dl.reshape(-1, 16).T
            wexp[16 * g:16 * (g + 1), :] = np.broadcast_to(
                wl[None, :], (16, L_STREAM)).astype(ml_dtypes.bfloat16)

        in_maps.append({
            "ft": ftc, "fn": fnc, "wmat": wmat,
            "esrc": esrc, "edst": edst, "wexp": wexp,
        })
    return in_maps


def _selu(x):
    alpha = 1.6732632423543772848170429916717
    scale = 1.0507009873554804934193349852946
    return (scale * np.where(x > 0, x, alpha * (np.exp(x) - 1))).astype(np.float32)


def _postprocess(results):
    S = np.concatenate([results[c]["s_out"][:NPC] for c in range(NCORES)], axis=0)
    U = np.sum([results[c]["u_out"].astype(np.float64) for c in range(NCORES)], axis=0)
    pv_all = np.sum([results[c]["pv"].astype(np.float64) for c in range(NCORES)],
                    axis=0)[:, 0]
    tg = float(np.sum([results[c]["ptg"].astype(np.float64) for c in range(NCORES)]))

    v = pv_all.reshape(8, 16).sum(axis=0)            # [16]
    m = float(v.sum())                               # sum of edge weights
    cs = S.astype(np.float64).sum(axis=0)            # cluster sizes [16]

    features_pooled = _selu((U / cs[:, None]).astype(np.float32))
    spectral = np.float32(-(tg - float(v @ v) / (2.0 * m)) / (2.0 * m))
    collapse = np.float32(0.1 * (np.linalg.norm(cs) / N * np.sqrt(K) - 1.0))
    return features_pooled, S.astype(np.float32), spectral, collapse


def kernel(features, edge_src, edge_dst, edge_weight, W, b):
    runner = _get_runner()
    in_maps = _prep_inputs(features, edge_src, edge_dst, edge_weight, W, b)
    results = runner(runner.concat_inputs(in_maps))
    return _postprocess(results)


# revision 15
# speedup vs baseline: 12.7179x; 1.1780x over previous
"""DMoN (graph pooling) kernel for 8 Trainium2 NeuronCores.

Full inputs -> full outputs. Internally:
  - nodes sharded 8 ways (12500/core); edges sharded by dst-owner core
  - phase A (per core): S = softmax(F@W+b) on own rows; U = S^T F partial
  - AllGather of S^T (f32) across cores
  - phase B (per core): edge reductions tg = sum w*<S[src],S[dst]>,
    v = sum w*S[dst] via GPSIMD ap_gather on SBUF-resident S tables
  - host: tiny final reductions (K=16 / KxD matrices and scalars)

Outputs mirror reference(): (features_pooled, assignments, spectral_loss, collapse_loss)
"""
import sys

sys.path.insert(0, "/opt/trn_rl_repo")
sys.path.insert(0, "/opt/trn_rl_repo/concourse")

import numpy as np
import ml_dtypes

from concourse import bacc, bass, mybir
from concourse.tile import TileContext
from concourse.bass_utils import run_bass_kernel_spmd
from concourse.masks import make_identity

NCORES = 8
N = 100000
E = 3200000
D = 384
K = 16
NPC = N // NCORES          # 12500 nodes per core
NTILES = (NPC + 127) // 128  # 98
NPAD = NTILES * 128        # 12544
EPC = E // NCORES          # 400000 edges per core
NI_CHUNK = 3200            # ap_gather indices per core-group per call
# per-group stream length, padded. 8 groups; expected 50000; slack for imbalance
L_STREAM = 52800           # multiple of NI_CHUNK*... (52800 = 16.5 chunks) -> use 53760?
# make it a multiple of NI_CHUNK and 16:
L_STREAM = ((EPC // 8 + 4000 + NI_CHUNK - 1) // NI_CHUNK) * NI_CHUNK  # 54400->  17 chunks
N_CHUNKS = L_STREAM // NI_CHUNK

F32 = mybir.dt.float32
BF16 = mybir.dt.bfloat16
I16 = mybir.dt.int16


def build_kernel():
    nc = bacc.Bacc("TRN2", debug=False, num_devices=NCORES)

    ft = nc.dram_tensor("ft", (D, NPAD), F32, kind="ExternalInput")       # F^T shard
    fn = nc.dram_tensor("fn", (NPAD, D), F32, kind="ExternalInput")       # F shard
    wmat = nc.dram_tensor("wmat", (D + 1, K), F32, kind="ExternalInput")  # [W; b]
    esrc = nc.dram_tensor("esrc", (128, L_STREAM // 16), I16, kind="ExternalInput")
    edst = nc.dram_tensor("edst", (128, L_STREAM // 16), I16, kind="ExternalInput")
    wexp = nc.dram_tensor("wexp", (128, L_STREAM), BF16, kind="ExternalInput")

    s_out = nc.dram_tensor("s_out", (NPAD, K), F32, kind="ExternalOutput")
    u_out = nc.dram_tensor("u_out", (K, D), F32, kind="ExternalOutput")
    pv = nc.dram_tensor("pv", (128, 1), F32, kind="ExternalOutput")
    ptg = nc.dram_tensor("ptg", (128, 1), F32, kind="ExternalOutput")

    HALF = (NTILES // 2) * 128  # 6272 = NPAD/2
    cc_inA = nc.dram_tensor("cc_inA", (K, HALF), BF16)                    # local S^T halves
    cc_inB = nc.dram_tensor("cc_inB", (K, NPAD - HALF), BF16)
    cc_outA = nc.dram_tensor("cc_outA", (NCORES * K, HALF), BF16, addr_space="Shared")
    cc_outB = nc.dram_tensor("cc_outB", (NCORES * K, NPAD - HALF), BF16, addr_space="Shared")

    with TileContext(nc) as tc:
        # ---------------- Phase A ----------------
        with tc.tile_pool(name="pa", bufs=4) as pa, \
             tc.tile_pool(name="pa1", bufs=1) as pa1, \
             tc.tile_pool(name="papsum", bufs=3, space="PSUM") as pap, \
             tc.tile_pool(name="paupsum", bufs=1, space="PSUM") as paup:
            wsb = pa1.tile([128, 3 * K], F32)
            for c in range(3):
                nc.sync.dma_start(out=wsb[:, c * K:(c + 1) * K],
                                  in_=wmat.ap()[c * 128:(c + 1) * 128, :])
            brow = pa1.tile([1, K], F32)
            nc.sync.dma_start(out=brow[:, :], in_=wmat.ap()[D:D + 1, :])
            onesb = pa1.tile([1, 128], F32)
            nc.vector.memset(onesb[:, :], 1.0)
            ident = pa1.tile([128, 128], F32)
            make_identity(nc, ident)

            u_psum = paup.tile([K, D], F32, space="PSUM")
            s_all = pa1.tile([128, NTILES * K], F32)   # staging for s_out

            for i in range(NTILES):
                ftl = pa.tile([128, 3 * 128], F32, tag="ftl")
                for c in range(3):
                    nc.sync.dma_start(
                        out=ftl[:, c * 128:(c + 1) * 128],
                        in_=ft.ap()[c * 128:(c + 1) * 128, i * 128:(i + 1) * 128])
                fnl = pa.tile([128, D], F32, tag="fnl")
                nc.sync.dma_start(out=fnl[:, :], in_=fn.ap()[i * 128:(i + 1) * 128, :])

                logits = pap.tile([128, K], F32, space="PSUM", tag="logits")
                for c in range(3):
                    nc.tensor.matmul(logits[:, :], lhsT=ftl[:, c * 128:(c + 1) * 128],
                                     rhs=wsb[:, c * K:(c + 1) * K],
                                     start=(c == 0), stop=False)
                nc.tensor.matmul(logits[:, :], lhsT=onesb[:1, :], rhs=brow[:1, :],
                                 start=False, stop=True)

                # logits are O(+-6) for this problem scale: exp is safe in f32
                # without the max-subtraction trick.
                e_t = pa.tile([128, K], F32, tag="e_t")
                ssum = pa.tile([128, 1], F32, tag="ssum")
                nc.scalar.activation(e_t[:, :], logits[:, :],
                                     mybir.ActivationFunctionType.Exp,
                                     scale=1.0, accum_out=ssum[:, :1])
                rsum = pa.tile([128, 1], F32, tag="rsum")
                nc.vector.reciprocal(rsum[:, :], ssum[:, :])
                s_t = pa.tile([128, K], F32, tag="s_t")
                nc.vector.tensor_scalar_mul(s_t[:, :], e_t[:, :], rsum[:, :1])

                # stage S rows for one big DMA later
                nc.vector.tensor_copy(s_all[:, i * K:(i + 1) * K], s_t[:, :])

                # U += S_tile^T @ F_tile
                nc.tensor.matmul(u_psum[:, :], lhsT=s_t[:, :], rhs=fnl[:, :],
                                 start=(i == 0), stop=(i == NTILES - 1))

                # S^T chunk -> cc_in
                st_ps = pap.tile([K, 128], F32, space="PSUM", tag="st_ps")
                nc.tensor.transpose(out=st_ps[:, :], in_=s_t[:, :], identity=ident[:, :])
                st_sb = pa.tile([K, 128], BF16, tag="st_sb")
                nc.vector.tensor_copy(st_sb[:, :], st_ps[:, :])
                if i < NTILES // 2:
                    nc.sync.dma_start(out=cc_inA.ap()[:, i * 128:(i + 1) * 128],
                                      in_=st_sb[:, :])
                else:
                    j = i - NTILES // 2
                    nc.sync.dma_start(out=cc_inB.ap()[:, j * 128:(j + 1) * 128],
                                      in_=st_sb[:, :])

                if i == NTILES // 2 - 1:
                    # first half of S^T is complete: overlap its AllGather
                    # with the remaining phase-A tiles
                    nc.gpsimd.collective_compute(
                        "AllGather", mybir.AluOpType.bypass,
                        replica_groups=[list(range(NCORES))],
                        ins=[cc_inA.ap()[:, :]],
                        outs=[cc_outA.ap()[:, :]],
                    )

            u_sb = pa1.tile([K, D], F32)
            nc.vector.tensor_copy(u_sb[:, :], u_psum[:, :])
            nc.sync.dma_start(out=u_out.ap()[:, :], in_=u_sb[:, :])
            # s_out: [NPAD, K] viewed [NTILES, 128, K]; s_all is [128, NTILES, K]
            nc.sync.dma_start(
                out=s_out.ap().rearrange("(t p) k -> p t k", p=128)[:, :, :],
                in_=s_all[:, :].rearrange("p (t k) -> p t k", k=K)[:, :, :])

            nc.gpsimd.collective_compute(
                "AllGather", mybir.AluOpType.bypass,
                replica_groups=[list(range(NCORES))],
                ins=[cc_inB.ap()[:, :]],
                outs=[cc_outB.ap()[:, :]],
            )

        # ---------------- Phase B ----------------
        with tc.tile_pool(name="pb", bufs=2) as pb, \
             tc.tile_pool(name="pb1", bufs=1) as pb1:
            # bf16 over the wire; ap_gather tables must be f32 -> cast in chunks
            sgrp = pb1.tile([128, NPAD], F32)   # src table: group g -> S^T of group g
            srep = pb1.tile([128, NPAD], F32)   # dst table: own S^T replicated x8
            CSTCH = NPAD // 4
            halves = [(cc_outA, cc_inA), (cc_outB, cc_inB)]
            for q in range(4):
                co, ci = halves[q // 2]
                off = (q % 2) * CSTCH
                stg = pb.tile([128, CSTCH], BF16, tag="stg")
                nc.sync.dma_start(out=stg[:, :], in_=co.ap()[:, off:off + CSTCH])
                nc.vector.tensor_copy(sgrp[:, q * CSTCH:(q + 1) * CSTCH], stg[:, :])
            for q in range(4):
                co, ci = halves[q // 2]
                off = (q % 2) * CSTCH
                stg = pb.tile([128, CSTCH], BF16, tag="stg")
                for g in range(NCORES):
                    nc.sync.dma_start(out=stg[g * K:(g + 1) * K, :],
                                      in_=ci.ap()[:, off:off + CSTCH])
                nc.vector.tensor_copy(srep[:, q * CSTCH:(q + 1) * CSTCH], stg[:, :])

            acc_v = pb1.tile([128, NI_CHUNK], F32)
            acc_t = pb1.tile([128, NI_CHUNK], F32)
            nc.vector.memset(acc_v[:, :], 0.0)
            nc.vector.memset(acc_t[:, :], 0.0)

            for ch in range(N_CHUNKS):
                i0 = ch * (NI_CHUNK // 16)
                i1 = (ch + 1) * (NI_CHUNK // 16)
                ix_s = pb.tile([128, NI_CHUNK // 16], I16, tag="ix_s")
                nc.sync.dma_start(out=ix_s[:, :], in_=esrc.ap()[:, i0:i1])
                ix_d = pb.tile([128, NI_CHUNK // 16], I16, tag="ix_d")
                nc.sync.dma_start(out=ix_d[:, :], in_=edst.ap()[:, i0:i1])
                wch = pb.tile([128, NI_CHUNK], BF16, tag="wch")
                nc.sync.dma_start(out=wch[:, :],
                                  in_=wexp.ap()[:, ch * NI_CHUNK:(ch + 1) * NI_CHUNK])

                gs = pb.tile([128, NI_CHUNK], F32, tag="gs")
                nc.gpsimd.ap_gather(gs[:, :], sgrp[:, :], ix_s[:, :],
                                    channels=128, num_elems=NPAD, d=1,
                                    num_idxs=NI_CHUNK)
                gd = pb.tile([128, NI_CHUNK], F32, tag="gd")
                nc.gpsimd.ap_gather(gd[:, :], srep[:, :], ix_d[:, :],
                                    channels=128, num_elems=NPAD, d=1,
                                    num_idxs=NI_CHUNK)

                # in-place: gd <- gd*w ; acc_v += gd ; gs <- gs*gd ; acc_t += gs
                nc.vector.tensor_tensor(out=gd[:, :], in0=gd[:, :], in1=wch[:, :],
                                        op=mybir.AluOpType.mult)
                nc.vector.tensor_tensor(out=acc_v[:, :], in0=acc_v[:, :], in1=gd[:, :],
                                        op=mybir.AluOpType.add)
                nc.vector.tensor_tensor(out=gs[:, :], in0=gs[:, :], in1=gd[:, :],
                                        op=mybir.AluOpType.mult)
                nc.vector.tensor_tensor(out=acc_t[:, :], in0=acc_t[:, :], in1=gs[:, :],
                                        op=mybir.AluOpType.add)

            pv_sb = pb1.tile([128, 1], F32)
            nc.vector.tensor_reduce(pv_sb[:, :], acc_v[:, :],
                                    axis=mybir.AxisListType.X, op=mybir.AluOpType.add)
            nc.sync.dma_start(out=pv.ap()[:, :], in_=pv_sb[:, :])
            ptg_sb = pb1.tile([128, 1], F32)
            nc.vector.tensor_reduce(ptg_sb[:, :], acc_t[:, :],
                                    axis=mybir.AxisListType.X, op=mybir.AluOpType.add)
            nc.sync.dma_start(out=ptg.ap()[:, :], in_=ptg_sb[:, :])

    nc.compile()
    return nc


_NC_CACHE = None
_RUNNER_CACHE = None


def _get_nc():
    global _NC_CACHE
    if _NC_CACHE is None:
        _NC_CACHE = build_kernel()
    return _NC_CACHE


def _make_runner(nc):
    """Reusable jitted SPMD runner (mirrors bass2jax.run_bass_via_pjrt, but
    caches the jit across calls and accepts pre-uploaded device arrays)."""
    import jax
    from jax.sharding import Mesh, PartitionSpec
    from jax.experimental.shard_map import shard_map
    from concourse import bass2jax, mybir as mb
    from concourse.bass2jax import _bass_exec_p, partition_id_tensor

    bass2jax.install_neuronx_cc_hook()

    partition_name = nc.partition_id_tensor.name if nc.partition_id_tensor else None
    in_names, out_names, out_avals, zero_shapes = [], [], [], []
    for alloc in nc.m.functions[0].allocations:
        if not isinstance(alloc, mb.MemoryLocationSet):
            continue
        name = alloc.memorylocations[0].name
        if alloc.kind == "ExternalInput":
            if name != partition_name:
                in_names.append(name)
        elif alloc.kind == "ExternalOutput":
            out_names.append(name)
            shape = tuple(alloc.tensor_shape)
            dtype = mb.dt.np(alloc.dtype)
            out_avals.append(jax.core.ShapedArray(shape, dtype))
            zero_shapes.append((shape, dtype))
    n_params = len(in_names)
    n_outs = len(out_avals)
    all_in_names = list(in_names) + list(out_names)
    if partition_name is not None:
        all_in_names.append(partition_name)
    donate = tuple(range(n_params, n_params + n_outs))

    def _body(*args):
        operands = list(args)
        if partition_name is not None:
            operands.append(partition_id_tensor())
        outs = _bass_exec_p.bind(
            *operands,
            out_avals=tuple(out_avals),
            in_names=tuple(all_in_names),
            out_names=tuple(out_names),
            lowering_input_output_aliases=(),
            sim_require_finite=True,
            sim_require_nnan=True,
            nc=nc,
        )
        return tuple(outs)

    devices = jax.devices()[:NCORES]
    mesh = Mesh(np.asarray(devices), ("core",))
    sharded = jax.jit(
        shard_map(_body, mesh=mesh,
                  in_specs=(PartitionSpec("core"),) * (n_params + n_outs),
                  out_specs=(PartitionSpec("core"),) * n_outs,
                  check_rep=False),
        donate_argnums=donate, keep_unused=True,
    )

    def concat_inputs(in_maps):
        return [np.concatenate([np.asarray(m[name]) for m in in_maps], axis=0)
                for name in in_names]

    def make_zeros():
        return [np.zeros((NCORES * s[0], *s[1:]), d) for s, d in zero_shapes]

    def run(concat_in):
        out_arrs = sharded(*concat_in, *make_zeros())
        import jax as _jax
        _jax.block_until_ready(out_arrs)
        return [
            {name: np.asarray(out_arrs[i]).reshape(NCORES, *out_avals[i].shape)[c]
             for i, name in enumerate(out_names)}
            for c in range(NCORES)
        ]

    run.concat_inputs = concat_inputs
    run.make_zeros = make_zeros
    run.sharded = sharded
    return run


def _get_runner():
    global _RUNNER_CACHE
    if _RUNNER_CACHE is None:
        _RUNNER_CACHE = _make_runner(_get_nc())
    return _RUNNER_CACHE


def _prep_inputs(features, edge_src, edge_dst, edge_weight, W, b):
    features = np.asarray(features, dtype=np.float32)
    edge_src = np.asarray(edge_src).astype(np.int64)
    edge_dst = np.asarray(edge_dst).astype(np.int64)
    edge_weight = np.asarray(edge_weight, dtype=np.float32)
    W = np.asarray(W, dtype=np.float32)
    b = np.asarray(b, dtype=np.float32)

    wmat = np.concatenate([W, b[None, :]], axis=0)  # [385, 16]

    in_maps = []
    FT = np.ascontiguousarray(features.T)  # [384, 100000]
    owner = edge_dst // NPC                # dst-owner core per edge
    for c in range(NCORES):
        r0, r1 = c * NPC, (c + 1) * NPC
        ftc = np.zeros((D, NPAD), dtype=np.float32)
        ftc[:, :NPC] = FT[:, r0:r1]
        fnc = np.zeros((NPAD, D), dtype=np.float32)
        fnc[:NPC, :] = features[r0:r1, :]

        sel = owner == c
        es = edge_src[sel]
        ed = edge_dst[sel]
        ew = edge_weight[sel]
        # bucket by src group
        g_s = es // NPC
        src_loc = (es % NPC).astype(np.int16)
        dst_loc = (ed % NPC).astype(np.int16)

        esrc = np.zeros((128, L_STREAM // 16), dtype=np.int16)
        edst = np.zeros((128, L_STREAM // 16), dtype=np.int16)
        wexp = np.zeros((128, L_STREAM), dtype=ml_dtypes.bfloat16)
        for g in range(NCORES):
            gsel = g_s == g
            n_g = int(gsel.sum())
            assert n_g <= L_STREAM, f"bucket overflow core {c} group {g}: {n_g}"
            sl = np.zeros(L_STREAM, dtype=np.int16)
            dl = np.zeros(L_STREAM, dtype=np.int16)
            wl = np.zeros(L_STREAM, dtype=np.float32)
            sl[:n_g] = src_loc[gsel]
            dl[:n_g] = dst_loc[gsel]
            wl[:n_g] = ew[gsel]
            # wrap [L] -> [L//16, 16] -> partitions 16g..16g+16 hold transposed
            esrc[16 * g:16 * (g + 1), :] = sl.reshape(-1, 16).T
            edst[16 * g:16 * (g + 1), :] = dl.reshape(-1, 16).T
            wexp[16 * g:16 * (g + 1), :] = np.broadcast_to(
                wl[None, :], (16, L_STREAM)).astype(ml_dtypes.bfloat16)

        in_maps.append({
            "ft": ftc, "fn": fnc, "wmat": wmat,
            "esrc": esrc, "edst": edst, "wexp": wexp,
        })
    return in_maps


def _selu(x):
    alpha = 1.6732632423543772848170429916717
    scale = 1.0507009873554804934193349852946
    return (scale * np.where(x > 0, x, alpha * (np.exp(x) - 1))).astype(np.float32)


def _postprocess(results):
    S = np.concatenate([results[c]["s_out"][:NPC] for c in range(NCORES)], axis=0)
    U = np.sum([results[c]["u_out"].astype(np.float64) for c in range(NCORES)], axis=0)
    pv_all = np.sum([results[c]["pv"].astype(np.float64) for c in range(NCORES)],
                    axis=0)[:, 0]
    tg = float(np.sum([results[c]["ptg"].astype(np.float64) for c in range(NCORES)]))

    v = pv_all.reshape(8, 16).sum(axis=0)            # [16]
    m = float(v.sum())                               # sum of edge weights
    cs = S.astype(np.float64).sum(axis=0)            # cluster sizes [16]

    features_pooled = _selu((U / cs[:, None]).astype(np.float32))
    spectral = np.float32(-(tg - float(v @ v) / (2.0 * m)) / (2.0 * m))
    collapse = np.float32(0.1 * (np.linalg.norm(cs) / N * np.sqrt(K) - 1.0))
    return features_pooled, S.astype(np.float32), spectral, collapse


def kernel(features, edge_src, edge_dst, edge_weight, W, b):
    runner = _get_runner()
    in_maps = _prep_inputs(features, edge_src, edge_dst, edge_weight, W, b)
    results = runner(runner.concat_inputs(in_maps))
    return _postprocess(results)
